# revision 29
# baseline (speedup 1.0000x reference)
"""Trainium2 Bass kernel for nn_MultiHeadAttention_77232101917088.

Causal MHA where only the LAST token's projected output is returned:
    out = (softmax_causal(q k^T / sqrt(hd)) v)[:, -1, :] @ Wo + bo

Only the last query row survives, so the problem collapses (the last
causal row attends to every position):
    q[b,:]        = x[b,-1,:] @ Wq
    u[b,h,d]      = sum_e Wk[d, h*128+e] * q[b, h*128+e]
    scores[b,h,j] = sum_d x[b,j,d] * u[b,h,d]           (no K/V materialized)
    p             = softmax_j(scores * 1/sqrt(hd))
    w[b,h,d]      = sum_j p[b,h,j] * x[b,j,d]
    ctx[b, hs]    = w[b,h,:] @ Wv[:, hs]
    out           = ctx @ Wo + bo

Sharding: ZERO collectives (measured ~100-130us exec for even a tiny
AllGather+ReduceScatter on this stack, not amortizing across runs).
Each core owns one batch and 4 heads (b = core//4, head group =
core%4); the host sums the 4 output partials per batch.

Design notes (measured on HW, v6 ~82us vs 108us baseline):
 - ONE deep DMA ring (sync HWDGE) carries all 21 MB in exact
   consumption order [wq, wkT, xT chunks + xn quarters, wv, wo].  A
   single queue measured 420-432 GB/s sustained vs ~310-370 for two
   parallel rings, and - critically - the Tile scheduler's static
   per-engine order is built with a ~368 GB/s/queue cost model, so a
   single FIFO ring keeps the modeled and real arrival ORDER in sync
   (two rings caused 9-17us head-of-line stalls on the PE stream).
 - The w-path copy of x (xn layout) is fp8 E3M4: its error enters the
   output linearly (~1.2e-2 rel, vs 2e-2 budget) and NOT through the
   softmax exponent.  The scores-path copy (xT) stays bf16 - exponent
   errors are NOT smoothed by this peaked softmax (measured).  Mixed
   bf16-lhsT x fp8-rhs matmuls verified exact on HW.  25.2 -> 21 MB.
 - On-chip PE transposition of bulk x was tried and REVERTED: LDW +
   128-free matmul + PSUM evac costs ~0.5us/tile in practice (cost
   model says 128 cycles); 128 tiles serialized 60us of tail.
 - PE warm-up: 16 throwaway matmuls while wq streams, so q runs at
   2.4 GHz instead of the HAM-throttled 1.2 GHz cold state.
 - exp is folded into the eT PSUM evacuation (scalar ACT, scale =
   1/sqrt(hd)); the z exp+accum feeds only rz, which is applied
   per-head (per-partition scalar) at the ctx evacuation - the whole
   softmax-denominator chain sits OFF the w -> wT -> ctx -> out path.
   w is kept unnormalized in bf16 (scale-free).
 - Half-0 w accumulation is j-outer (overlaps the stream); half-1 is
   oc-outer so the four w chunks complete sequentially and their
   evac / wT transpose / ctx accumulation pipeline behind them.  out
   accumulates per 512-chunk (wo chunk-major) with the bias as a
   final rank-1 accumulation step, and each out chunk DMAs as it
   completes.  Max-subtraction is skipped (|scores*ISCALE| < ~5 for
   this input class).
"""

import numpy as np
from ml_dtypes import bfloat16, float8_e3m4

import concourse.bacc as bacc
import concourse.bass as bass
import concourse.mybir as mybir
import concourse.tile as tile
from concourse.bass_utils import run_bass_kernel_spmd

P = 128          # partitions
B = 2            # batch
S = 2048         # sequence length
D = 2048         # model dim
NH = 16          # heads
HD = 128         # head dim
NC = 8           # cores
HPC = 4          # heads per core
HW = HPC * HD    # per-core head-column width (512)
DT = D // P      # depth subtiles (16)
JT = S // P      # sequence subtiles (16)
NJC = 4          # 512-wide chunks
JC = S // NJC    # 512
HJ = S // 2      # j-half width (1024)
QT = 4           # j-subtiles per xn quarter
ISCALE = 1.0 / np.sqrt(HD)

FP32 = mybir.dt.float32
BF16 = mybir.dt.bfloat16
F8E3 = mybir.dt.float8e3


def _build_program():
    nc = bacc.Bacc(
        "TRN2",
        target_bir_lowering=False,
        debug=False,
        enable_asserts=False,
        num_devices=NC,
    )

    # ---- per-core DRAM inputs (host pre-arranged, contiguous loads) ------
    xlastT = nc.dram_tensor("xlastT", [P, DT], BF16, kind="ExternalInput").ap()
    ident = nc.dram_tensor("ident", [HPC, HPC], BF16, kind="ExternalInput").ap()
    wq = nc.dram_tensor("wq", [P, DT, HW], BF16, kind="ExternalInput").ap()
    wkT = nc.dram_tensor("wkT", [P, HPC, D], BF16, kind="ExternalInput").ap()
    xtc = [nc.dram_tensor(f"xtc{c}", [P, DT, JC], BF16, kind="ExternalInput").ap()
           for c in range(NJC)]
    xnq = [nc.dram_tensor(f"xnq{i}", [P, QT, D], F8E3, kind="ExternalInput").ap()
           for i in range(4)]
    wvp = [nc.dram_tensor(f"wvp{i}", [P, DT, 2 * HD], BF16, kind="ExternalInput").ap()
           for i in range(2)]
    wop = [nc.dram_tensor(f"wop{i}", [P, 2, HPC, JC], BF16, kind="ExternalInput").ap()
           for i in range(2)]
    bo_sh = nc.dram_tensor("bo_sh", [D], BF16, kind="ExternalInput").ap()

    out_sh = nc.dram_tensor("out_sh", [1, D], FP32, kind="ExternalOutput").ap()

    with tile.TileContext(nc) as tc:
        with (
            tc.tile_pool(name="persist", bufs=1) as pp,
            tc.tile_pool(name="work", bufs=1) as wp,
            tc.tile_pool(name="psA", bufs=2, space="PSUM") as psA,
            tc.tile_pool(name="psW", bufs=1, space="PSUM") as psW,
            tc.tile_pool(name="psB", bufs=2, space="PSUM") as psB,
        ):
            # ---- tiny sync-ring loads -----------------------------------
            xlastT_sb = pp.tile([P, DT], BF16, name="xlastT_sb")
            nc.sync.dma_start(xlastT_sb[:], xlastT)
            ident_sb = pp.tile([HPC, HPC], BF16, name="ident_sb")
            nc.sync.dma_start(ident_sb[:], ident)
            # bias rides partition 0 of a zeroed tile; a unit-vector lhsT
            # turns the bias add into one extra matmul accumulation step.
            bo_sb = pp.tile([P, D], BF16, name="bo_sb")
            nc.vector.memset(bo_sb[:], 0.0)
            nc.sync.dma_start(bo_sb[0:1, :], bo_sh.rearrange("(o m) -> o m", o=1))
            e0_sb = pp.tile([P, 1], BF16, name="e0_sb")
            nc.vector.memset(e0_sb[:], 0.0)
            nc.vector.memset(e0_sb[0:1, 0:1], 1.0)
            # PE warm-up: the HAM clock gate holds the PE at 1.2 GHz until
            # ~3.4us of sustained activity.  While waiting for wq (~13us on
            # the ring) run throwaway matmuls so q and everything after run
            # at 2.4 GHz.  Results are never read.
            warm_sb = pp.tile([P, JC], BF16, name="warm_sb")
            nc.vector.memset(warm_sb[:], 0.0)
            for wi in range(16):
                ps_wu = psB.tile([1, JC], FP32, name="ps_wu", tag="psB")
                nc.tensor.matmul(
                    ps_wu[:], lhsT=warm_sb[:, 0:1], rhs=warm_sb[:],
                    start=True, stop=True,
                )

            # ---- bulk loads: ONE sync-HWDGE ring, consumption order -----
            # wq, wkT, xtc0, xtc1, xnq0, xnq1, xtc2, xtc3, xnq2, xnq3,
            # wv01, wv23, wo01, wo23   (~21 MB total)
            wq_sb = pp.tile([P, DT, HW], BF16, name="wq_sb")
            nc.sync.dma_start(wq_sb[:], wq)
            wkT_sb = pp.tile([P, HPC, D], BF16, name="wkT_sb")
            nc.sync.dma_start(wkT_sb[:], wkT)
            xtc_sb = [pp.tile([P, DT, JC], BF16, name=f"xtc_sb{c}")
                      for c in range(NJC)]
            xnq_sb = [pp.tile([P, QT, D], F8E3, name=f"xnq_sb{i}")
                      for i in range(4)]
            nc.sync.dma_start(xtc_sb[0][:], xtc[0])
            nc.sync.dma_start(xtc_sb[1][:], xtc[1])
            nc.sync.dma_start(xnq_sb[0][:], xnq[0])
            nc.sync.dma_start(xnq_sb[1][:], xnq[1])
            nc.sync.dma_start(xtc_sb[2][:], xtc[2])
            nc.sync.dma_start(xtc_sb[3][:], xtc[3])
            nc.sync.dma_start(xnq_sb[2][:], xnq[2])
            nc.sync.dma_start(xnq_sb[3][:], xnq[3])
            # Wv (head-major) aliases Wq's buffer; Wo (chunk-major) aliases
            # WkT's — both originals are fully consumed by ~15us.
            wv_sb = pp.tile([P, 2, DT, 2 * HD], BF16, name="wv_sb",
                            tag="wq_sb")
            nc.sync.dma_start(wv_sb[:, 0, :, :], wvp[0])
            nc.sync.dma_start(wv_sb[:, 1, :, :], wvp[1])
            wo_sb = pp.tile([P, NJC, HPC, JC], BF16, name="wo_sb",
                            tag="wkT_sb")
            nc.sync.dma_start(wo_sb[:, 0:2, :, :], wop[0])
            nc.sync.dma_start(wo_sb[:, 2:4, :, :], wop[1])

            # ---- A: q = xlast @ Wq[:, hs]  ([1, 512]) -------------------
            ps_q = psB.tile([1, HW], FP32, name="ps_q", tag="psB")
            for t in range(DT):
                nc.tensor.matmul(
                    ps_q[:],
                    lhsT=xlastT_sb[:, t:t + 1],
                    rhs=wq_sb[:, t, :],
                    start=(t == 0),
                    stop=(t == DT - 1),
                )
            q_sb = wp.tile([1, HW], BF16, name="q_sb")
            nc.vector.tensor_copy(q_sb[:], ps_q[:])
            qT_sb = wp.tile([P, HPC], BF16, name="qT_sb")
            for es in range(HPC):
                ps_qt = psB.tile([P, 1], BF16, name="ps_qt", tag="psB")
                nc.tensor.transpose(
                    ps_qt[:], q_sb[:, es * P:(es + 1) * P], ident_sb[:1, :1]
                )
                nc.vector.tensor_copy(qT_sb[:, es:es + 1], ps_qt[:])
            qtil_sb = wp.tile([P, HPC, HPC], BF16, name="qtil_sb")
            nc.vector.memset(qtil_sb[:], 0.0)
            for es in range(HPC):
                nc.vector.tensor_copy(
                    qtil_sb[:, es, es:es + 1], qT_sb[:, es:es + 1])

            # ---- B: u[h, d], then PE-transpose to uT[p, t, h] -----------
            u_sb = wp.tile([HPC, D], BF16, name="u_sb", tag="udw")
            for oc in range(NJC):
                ps_u = psB.tile([HPC, JC], FP32, name="ps_u", tag="psB")
                for es in range(HPC):
                    nc.tensor.matmul(
                        ps_u[:],
                        lhsT=qtil_sb[:, es, :],
                        rhs=wkT_sb[:, es, oc * JC:(oc + 1) * JC],
                        start=(es == 0),
                        stop=(es == HPC - 1),
                    )
                nc.vector.tensor_copy(u_sb[:, oc * JC:(oc + 1) * JC], ps_u[:])
            uT_sb = wp.tile([P, DT, HPC], BF16, name="uT_sb")
            for t in range(DT):
                ps_t = psB.tile([P, HPC], BF16, name="ps_ut", tag="psB")
                nc.tensor.transpose(
                    ps_t[:], u_sb[:, t * P:(t + 1) * P], ident_sb[:]
                )
                nc.vector.tensor_copy(uT_sb[:, t, :], ps_t[:])

            # ---- C+D interleaved: scores per j-half, then that half's ---
            # ---- w accumulation (fills the PE gap while the next xT  ----
            # ---- chunks stream in)                                   ----
            sc_sb = wp.tile([HPC, S], BF16, name="sc_sb", tag="udw")
            eT_sb = wp.tile([P, JT, HPC], BF16, name="eT_sb")
            z2_sb = wp.tile([HPC, 2], FP32, name="z2_sb")
            w_sb = wp.tile([HPC, D], BF16, name="w_sb", tag="udw")
            ps_w = [psW.tile([HPC, JC], FP32, name=f"ps_w{oc}", tag=f"psW{oc}")
                    for oc in range(NJC)]

            for half in range(2):
                for jc in (2 * half, 2 * half + 1):
                    ps_s = psA.tile([HPC, JC], FP32, name="ps_s", tag="psA")
                    for t in range(DT):
                        nc.tensor.matmul(
                            ps_s[:],
                            lhsT=uT_sb[:, t, :],
                            rhs=xtc_sb[jc][:, t, :],
                            start=(t == 0),
                            stop=(t == DT - 1),
                        )
                    nc.vector.tensor_copy(
                        sc_sb[:, jc * JC:(jc + 1) * JC], ps_s[:])
                for lt in range(JT // 2):
                    t = half * (JT // 2) + lt
                    ps_e = psB.tile([P, HPC], BF16, name="ps_e", tag="psB")
                    nc.tensor.transpose(
                        ps_e[:], sc_sb[:, t * P:(t + 1) * P], ident_sb[:]
                    )
                    # exp folded into the PSUM evacuation (scalar ACT)
                    nc.scalar.activation(
                        eT_sb[:, t, :], ps_e[:],
                        mybir.ActivationFunctionType.Exp, scale=float(ISCALE),
                    )
                # softmax denominator for this half via exp+accum over the
                # raw scores; feeds only rz, which is consumed at the ctx
                # evacuation, so this sits OFF the w critical chain.
                nc.scalar.activation(
                    sc_sb[:, half * HJ:(half + 1) * HJ],
                    sc_sb[:, half * HJ:(half + 1) * HJ],
                    mybir.ActivationFunctionType.Exp, scale=float(ISCALE),
                    accum_out=z2_sb[:, half:half + 1],
                )
                # w accumulation: half 0 j-outer (overlaps the stream);
                # half 1 is emitted below, oc-outer, so chunks complete
                # sequentially and evac/wT/ctx pipeline behind them.
                if half == 0:
                    for JJ in range(8):
                        for oc in range(NJC):
                            nc.tensor.matmul(
                                ps_w[oc][:],
                                lhsT=eT_sb[:, JJ, :],
                                rhs=xnq_sb[JJ // QT][:, JJ % QT,
                                                     oc * JC:(oc + 1) * JC],
                                start=(JJ == 0),
                                stop=False,
                            )
            rz_sb = wp.tile([HPC, 1], FP32, name="rz_sb")
            z_sb = wp.tile([HPC, 1], FP32, name="z_sb")
            nc.vector.tensor_tensor(
                z_sb[:], z2_sb[:, 0:1], z2_sb[:, 1:2], mybir.AluOpType.add)
            nc.vector.reciprocal(rz_sb[:], z_sb[:])

            # ---- D+E: half-1 w chunks pipelined into wT and ctx ---------
            # w kept UNNORMALIZED (bf16 is scale-free); 1/z applied per-head
            # at the ctx evacuation, keeping exp/z/rz off the w chain.
            wT_sb = wp.tile([P, DT, HPC], BF16, name="wT_sb")
            cf_sb = wp.tile([HPC, HW], BF16, name="cf_sb")
            ctxT_sb = wp.tile([P, HPC, 1], BF16, name="ctxT_sb")
            ps_c = [psA.tile([HPC, 2 * HD], FP32, name=f"ps_c{pr}",
                             tag="psA") for pr in range(2)]
            for oc in range(NJC):
                for JJ in range(8, JT):
                    nc.tensor.matmul(
                        ps_w[oc][:],
                        lhsT=eT_sb[:, JJ, :],
                        rhs=xnq_sb[JJ // QT][:, JJ % QT,
                                             oc * JC:(oc + 1) * JC],
                        start=False,
                        stop=(JJ == JT - 1),
                    )
                if oc % 2 == 0:
                    nc.vector.tensor_copy(
                        w_sb[:, oc * JC:(oc + 1) * JC], ps_w[oc][:])
                else:
                    nc.scalar.activation(
                        w_sb[:, oc * JC:(oc + 1) * JC], ps_w[oc][:],
                        mybir.ActivationFunctionType.Copy,
                    )
                # High priority: the scheduler otherwise batches ALL w
                # matmuls before any wT/ctx work; boosting these makes them
                # pop the moment their chunk's w evac lands, interleaving
                # the tail chain into the w stream.
                with tc.high_priority():
                    for lt in range(QT):
                        t = oc * QT + lt
                        ps_t = psB.tile([P, HPC], BF16, name="ps_wt",
                                        tag="psB")
                        nc.tensor.transpose(
                            ps_t[:], w_sb[:, t * P:(t + 1) * P], ident_sb[:]
                        )
                        nc.vector.tensor_copy(wT_sb[:, t, :], ps_t[:])
                    for pr in range(2):
                        for lt in range(QT):
                            t = oc * QT + lt
                            nc.tensor.matmul(
                                ps_c[pr][:],
                                lhsT=wT_sb[:, t, :],
                                rhs=wv_sb[:, pr, t, :],
                                start=(t == 0),
                                stop=(t == DT - 1),
                            )
            for pr in range(2):
                if pr == 0:
                    nc.vector.tensor_scalar_mul(
                        cf_sb[:, pr * 2 * HD:(pr + 1) * 2 * HD], ps_c[pr][:],
                        rz_sb[:])
                else:
                    nc.scalar.activation(
                        cf_sb[:, pr * 2 * HD:(pr + 1) * 2 * HD], ps_c[pr][:],
                        mybir.ActivationFunctionType.Copy, scale=rz_sb[:],
                    )
                for hh in range(2):
                    h = pr * 2 + hh
                    ps_ct = psB.tile([P, HPC], BF16, name="ps_ct", tag="psB")
                    nc.tensor.transpose(
                        ps_ct[:], cf_sb[:, h * HD:(h + 1) * HD], ident_sb[:]
                    )
                    nc.vector.tensor_copy(ctxT_sb[:, h, :], ps_ct[:, h:h + 1])

            # ---- F: out per 512-chunk as each Wo chunk arrives ----------
            # bias added as a 5th accumulation step (e0 . bo rank-1 matmul)
            o_sb = wp.tile([1, D], FP32, name="o_sb")
            for oc in range(NJC):
                ps_o = psA.tile([1, JC], FP32, name="ps_o", tag="psA")
                for sub in range(HPC):
                    nc.tensor.matmul(
                        ps_o[:],
                        lhsT=ctxT_sb[:, sub, :],
                        rhs=wo_sb[:, oc, sub, :],
                        start=(sub == 0),
                        stop=False,
                    )
                nc.tensor.matmul(
                    ps_o[:],
                    lhsT=e0_sb[:],
                    rhs=bo_sb[:, oc * JC:(oc + 1) * JC],
                    start=False,
                    stop=True,
                )
                eng = nc.vector if oc % 2 == 0 else nc.scalar
                if oc % 2 == 0:
                    eng.tensor_copy(o_sb[:, oc * JC:(oc + 1) * JC], ps_o[:])
                else:
                    eng.activation(
                        o_sb[:, oc * JC:(oc + 1) * JC], ps_o[:],
                        mybir.ActivationFunctionType.Copy,
                    )
                nc.sync.dma_start(
                    out_sh[:, oc * JC:(oc + 1) * JC],
                    o_sb[:, oc * JC:(oc + 1) * JC],
                )

    nc.compile()
    return nc


_PROGRAM = None


def _get_program():
    global _PROGRAM
    if _PROGRAM is None:
        _PROGRAM = _build_program()
    return _PROGRAM


def _shard_inputs(x, Wq, Wk, Wv, Wo, bo):
    xb = x.astype(bfloat16)
    # w-path copy of x in fp8 E3M4 (cast from f32; |x| < 15.5 always for
    # N(0,1) data).  Errors here enter the output linearly (~1.2e-2 rel),
    # NOT through the softmax exponent, so fp8 is safe on this path only.
    x8 = x.astype(float8_e3m4)
    wqb = Wq.astype(bfloat16)
    wkb = Wk.astype(bfloat16)
    wvb = Wv.astype(bfloat16)
    wob = Wo.astype(bfloat16)
    bo4 = (bo / HPC).astype(bfloat16)
    identity = np.eye(HPC, dtype=bfloat16)

    in_maps = []
    for core in range(NC):
        b = core // HPC
        hg = core % HPC
        hs = slice(hg * HW, (hg + 1) * HW)
        xlastT_pre = np.ascontiguousarray(xb[b, -1, :].reshape(DT, P).T)
        wq_pre = np.ascontiguousarray(
            wqb[:, hs].reshape(DT, P, HW).transpose(1, 0, 2))
        wkT_pre = np.ascontiguousarray(
            wkb[:, hs].T.reshape(HPC, P, D).transpose(1, 0, 2))
        xT_pre = xb[b].T.reshape(DT, P, S).transpose(1, 0, 2)
        xn8_pre = x8[b].reshape(JT, P, D).transpose(1, 0, 2)
        wv_pre = wvb[:, hs].reshape(DT, P, HW).transpose(1, 0, 2)
        wo_pre = wob[hs, :].reshape(HPC, P, D).transpose(1, 0, 2)
        m = {
            "xlastT": xlastT_pre,
            "ident": identity,
            "wq": wq_pre,
            "wkT": wkT_pre,
            "bo_sh": bo4,
        }
        for c in range(NJC):
            m[f"xtc{c}"] = np.ascontiguousarray(
                xT_pre[:, :, c * JC:(c + 1) * JC])
        for i in range(4):
            m[f"xnq{i}"] = np.ascontiguousarray(
                xn8_pre[:, i * QT:(i + 1) * QT, :])
        # [P, DT, 256] head-pair column blocks for ctx streaming
        m["wvp0"] = np.ascontiguousarray(wv_pre[:, :, 0:2 * HD])
        m["wvp1"] = np.ascontiguousarray(wv_pre[:, :, 2 * HD:4 * HD])
        # [P, 2, HPC, JC] chunk-major pairs for per-chunk out streaming
        wo_cm = np.ascontiguousarray(
            wo_pre.reshape(P, HPC, NJC, JC).transpose(0, 2, 1, 3))
        m["wop0"] = np.ascontiguousarray(wo_cm[:, 0:2])
        m["wop1"] = np.ascontiguousarray(wo_cm[:, 2:4])
        in_maps.append(m)
    return in_maps


def kernel(x, Wq, Wk, Wv, Wo, bo, _trace=False, _trace_cores=None):
    x = np.asarray(x, dtype=np.float32)
    Wq = np.asarray(Wq, dtype=np.float32)
    Wk = np.asarray(Wk, dtype=np.float32)
    Wv = np.asarray(Wv, dtype=np.float32)
    Wo = np.asarray(Wo, dtype=np.float32)
    bo = np.asarray(bo, dtype=np.float32)

    nc = _get_program()
    in_maps = _shard_inputs(x, Wq, Wk, Wv, Wo, bo)
    res = run_bass_kernel_spmd(
        nc, in_maps, core_ids=list(range(NC)),
        trace=_trace, trace_cores=_trace_cores,
    )
    out = np.zeros((B, D), dtype=np.float32)
    for core in range(NC):
        out[core // HPC] += res.results[core]["out_sh"][0]
    if _trace:
        kernel._last_results = res
    return out


# revision 30
# speedup vs baseline: 1.0196x; 1.0196x over previous
"""Trainium2 Bass kernel for nn_MultiHeadAttention_77232101917088.

Causal MHA where only the LAST token's projected output is returned:
    out = (softmax_causal(q k^T / sqrt(hd)) v)[:, -1, :] @ Wo + bo

Only the last query row survives, so the problem collapses (the last
causal row attends to every position):
    q[b,:]        = x[b,-1,:] @ Wq
    u[b,h,d]      = sum_e Wk[d, h*128+e] * q[b, h*128+e]
    scores[b,h,j] = sum_d x[b,j,d] * u[b,h,d]           (no K/V materialized)
    p             = softmax_j(scores * 1/sqrt(hd))
    w[b,h,d]      = sum_j p[b,h,j] * x[b,j,d]
    ctx[b, hs]    = w[b,h,:] @ Wv[:, hs]
    out           = ctx @ Wo + bo

Sharding: ZERO collectives (measured ~100-130us exec for even a tiny
AllGather+ReduceScatter on this stack, not amortizing across runs).
Each core owns one batch and 4 heads (b = core//4, head group =
core%4); the host sums the 4 output partials per batch.

Design notes (measured on HW, v6 ~82us vs 108us baseline):
 - ONE deep DMA ring (sync HWDGE) carries all 21 MB in exact
   consumption order [wq, wkT, xT chunks + xn quarters, wv, wo].  A
   single queue measured 420-432 GB/s sustained vs ~310-370 for two
   parallel rings, and - critically - the Tile scheduler's static
   per-engine order is built with a ~368 GB/s/queue cost model, so a
   single FIFO ring keeps the modeled and real arrival ORDER in sync
   (two rings caused 9-17us head-of-line stalls on the PE stream).
 - The w-path copy of x (xn layout) is fp8 E3M4: its error enters the
   output linearly (~1.2e-2 rel, vs 2e-2 budget) and NOT through the
   softmax exponent.  The scores-path copy (xT) stays bf16 - exponent
   errors are NOT smoothed by this peaked softmax (measured).  Mixed
   bf16-lhsT x fp8-rhs matmuls verified exact on HW.  25.2 -> 21 MB.
 - On-chip PE transposition of bulk x was tried and REVERTED: LDW +
   128-free matmul + PSUM evac costs ~0.5us/tile in practice (cost
   model says 128 cycles); 128 tiles serialized 60us of tail.
 - PE warm-up: 16 throwaway matmuls while wq streams, so q runs at
   2.4 GHz instead of the HAM-throttled 1.2 GHz cold state.
 - exp is folded into the eT PSUM evacuation (scalar ACT, scale =
   1/sqrt(hd)); the z exp+accum feeds only rz, which is applied
   per-head (per-partition scalar) at the ctx evacuation - the whole
   softmax-denominator chain sits OFF the w -> wT -> ctx -> out path.
   w is kept unnormalized in bf16 (scale-free).
 - Half-0 w accumulation is j-outer (overlaps the stream); half-1 is
   oc-outer so the four w chunks complete sequentially and their
   evac / wT transpose / ctx accumulation pipeline behind them.  out
   accumulates per 512-chunk (wo chunk-major) with the bias as a
   final rank-1 accumulation step, and each out chunk DMAs as it
   completes.  Max-subtraction is skipped (|scores*ISCALE| < ~5 for
   this input class).
"""

import numpy as np
from ml_dtypes import bfloat16, float8_e3m4

import concourse.bacc as bacc
import concourse.bass as bass
import concourse.mybir as mybir
import concourse.tile as tile
from concourse.bass_utils import run_bass_kernel_spmd

P = 128          # partitions
B = 2            # batch
S = 2048         # sequence length
D = 2048         # model dim
NH = 16          # heads
HD = 128         # head dim
NC = 8           # cores
HPC = 4          # heads per core
HW = HPC * HD    # per-core head-column width (512)
DT = D // P      # depth subtiles (16)
JT = S // P      # sequence subtiles (16)
NJC = 4          # 512-wide chunks
JC = S // NJC    # 512
HJ = S // 2      # j-half width (1024)
QT = 4           # j-subtiles per xn quarter
ISCALE = 1.0 / np.sqrt(HD)

FP32 = mybir.dt.float32
BF16 = mybir.dt.bfloat16
F8E3 = mybir.dt.float8e3


def _build_program():
    nc = bacc.Bacc(
        "TRN2",
        target_bir_lowering=False,
        debug=False,
        enable_asserts=False,
        num_devices=NC,
    )

    # ---- per-core DRAM inputs (host pre-arranged, contiguous loads) ------
    xlastT = nc.dram_tensor("xlastT", [P, DT], BF16, kind="ExternalInput").ap()
    ident = nc.dram_tensor("ident", [HPC, HPC], BF16, kind="ExternalInput").ap()
    wq = nc.dram_tensor("wq", [P, DT, HW], BF16, kind="ExternalInput").ap()
    wkT = nc.dram_tensor("wkT", [P, HPC, D], BF16, kind="ExternalInput").ap()
    xtc = [nc.dram_tensor(f"xtc{c}", [P, DT, JC], BF16, kind="ExternalInput").ap()
           for c in range(NJC)]
    xnq = [nc.dram_tensor(f"xnq{i}", [P, QT, D], F8E3, kind="ExternalInput").ap()
           for i in range(4)]
    wvp = [nc.dram_tensor(f"wvp{i}", [P, DT, 2 * HD], BF16, kind="ExternalInput").ap()
           for i in range(2)]
    wop = [nc.dram_tensor(f"wop{i}", [P, 2, HPC, JC], BF16, kind="ExternalInput").ap()
           for i in range(2)]
    bo_sh = nc.dram_tensor("bo_sh", [D], BF16, kind="ExternalInput").ap()

    out_sh = nc.dram_tensor("out_sh", [1, D], FP32, kind="ExternalOutput").ap()

    with tile.TileContext(nc) as tc:
        with (
            tc.tile_pool(name="persist", bufs=1) as pp,
            tc.tile_pool(name="work", bufs=1) as wp,
            tc.tile_pool(name="psA", bufs=2, space="PSUM") as psA,
            tc.tile_pool(name="psW", bufs=1, space="PSUM") as psW,
            tc.tile_pool(name="psB", bufs=2, space="PSUM") as psB,
        ):
            # ---- tiny sync-ring loads -----------------------------------
            xlastT_sb = pp.tile([P, DT], BF16, name="xlastT_sb")
            nc.sync.dma_start(xlastT_sb[:], xlastT)
            ident_sb = pp.tile([HPC, HPC], BF16, name="ident_sb")
            nc.sync.dma_start(ident_sb[:], ident)
            # bias rides partition 0 of a zeroed tile; a unit-vector lhsT
            # turns the bias add into one extra matmul accumulation step.
            bo_sb = pp.tile([P, D], BF16, name="bo_sb")
            nc.vector.memset(bo_sb[:], 0.0)
            nc.sync.dma_start(bo_sb[0:1, :], bo_sh.rearrange("(o m) -> o m", o=1))
            e0_sb = pp.tile([P, 1], BF16, name="e0_sb")
            nc.vector.memset(e0_sb[:], 0.0)
            nc.vector.memset(e0_sb[0:1, 0:1], 1.0)
            # PE warm-up: the HAM clock gate holds the PE at 1.2 GHz until
            # ~3.4us of sustained activity.  While waiting for wq (~13us on
            # the ring) run throwaway matmuls so q and everything after run
            # at 2.4 GHz.  Results are never read.
            warm_sb = pp.tile([P, JC], BF16, name="warm_sb")
            nc.vector.memset(warm_sb[:], 0.0)
            for wi in range(16):
                ps_wu = psB.tile([1, JC], FP32, name="ps_wu", tag="psB")
                nc.tensor.matmul(
                    ps_wu[:], lhsT=warm_sb[:, 0:1], rhs=warm_sb[:],
                    start=True, stop=True,
                )

            # ---- bulk loads: ONE sync-HWDGE ring, consumption order -----
            # wq, wkT, xtc0, xtc1, xnq0, xnq1, xtc2, xtc3, xnq2, xnq3,
            # wv01, wv23, wo01, wo23   (~21 MB total)
            wq_sb = pp.tile([P, DT, HW], BF16, name="wq_sb")
            nc.sync.dma_start(wq_sb[:], wq)
            wkT_sb = pp.tile([P, HPC, D], BF16, name="wkT_sb")
            nc.sync.dma_start(wkT_sb[:], wkT)
            xtc_sb = [pp.tile([P, DT, JC], BF16, name=f"xtc_sb{c}")
                      for c in range(NJC)]
            xnq_sb = [pp.tile([P, QT, D], F8E3, name=f"xnq_sb{i}")
                      for i in range(4)]
            nc.sync.dma_start(xtc_sb[0][:], xtc[0])
            nc.sync.dma_start(xtc_sb[1][:], xtc[1])
            nc.sync.dma_start(xnq_sb[0][:], xnq[0])
            nc.sync.dma_start(xnq_sb[1][:], xnq[1])
            nc.sync.dma_start(xtc_sb[2][:], xtc[2])
            nc.sync.dma_start(xtc_sb[3][:], xtc[3])
            nc.sync.dma_start(xnq_sb[2][:], xnq[2])
            nc.sync.dma_start(xnq_sb[3][:], xnq[3])
            # Wv (head-major) aliases Wq's buffer; Wo (chunk-major) aliases
            # WkT's — both originals are fully consumed by ~15us.
            wv_sb = pp.tile([P, 2, DT, 2 * HD], BF16, name="wv_sb",
                            tag="wq_sb")
            nc.sync.dma_start(wv_sb[:, 0, :, :], wvp[0])
            nc.sync.dma_start(wv_sb[:, 1, :, :], wvp[1])
            wo_sb = pp.tile([P, NJC, HPC, JC], BF16, name="wo_sb",
                            tag="wkT_sb")
            nc.sync.dma_start(wo_sb[:, 0:2, :, :], wop[0])
            nc.sync.dma_start(wo_sb[:, 2:4, :, :], wop[1])

            # ---- A: q = xlast @ Wq[:, hs]  ([1, 512]) -------------------
            ps_q = psB.tile([1, HW], FP32, name="ps_q", tag="psB")
            for t in range(DT):
                nc.tensor.matmul(
                    ps_q[:],
                    lhsT=xlastT_sb[:, t:t + 1],
                    rhs=wq_sb[:, t, :],
                    start=(t == 0),
                    stop=(t == DT - 1),
                )
            q_sb = wp.tile([1, HW], BF16, name="q_sb")
            nc.vector.tensor_copy(q_sb[:], ps_q[:])
            qT_sb = wp.tile([P, HPC], BF16, name="qT_sb")
            for es in range(HPC):
                ps_qt = psB.tile([P, 1], BF16, name="ps_qt", tag="psB")
                nc.tensor.transpose(
                    ps_qt[:], q_sb[:, es * P:(es + 1) * P], ident_sb[:1, :1]
                )
                nc.vector.tensor_copy(qT_sb[:, es:es + 1], ps_qt[:])
            qtil_sb = wp.tile([P, HPC, HPC], BF16, name="qtil_sb")
            nc.vector.memset(qtil_sb[:], 0.0)
            for es in range(HPC):
                nc.vector.tensor_copy(
                    qtil_sb[:, es, es:es + 1], qT_sb[:, es:es + 1])

            # ---- B: u[h, d], then PE-transpose to uT[p, t, h] -----------
            u_sb = wp.tile([HPC, D], BF16, name="u_sb", tag="udw")
            for oc in range(NJC):
                ps_u = psB.tile([HPC, JC], FP32, name="ps_u", tag="psB")
                for es in range(HPC):
                    nc.tensor.matmul(
                        ps_u[:],
                        lhsT=qtil_sb[:, es, :],
                        rhs=wkT_sb[:, es, oc * JC:(oc + 1) * JC],
                        start=(es == 0),
                        stop=(es == HPC - 1),
                    )
                nc.vector.tensor_copy(u_sb[:, oc * JC:(oc + 1) * JC], ps_u[:])
            uT_sb = wp.tile([P, DT, HPC], BF16, name="uT_sb")
            for t in range(DT):
                ps_t = psB.tile([P, HPC], BF16, name="ps_ut", tag="psB")
                nc.tensor.transpose(
                    ps_t[:], u_sb[:, t * P:(t + 1) * P], ident_sb[:]
                )
                nc.vector.tensor_copy(uT_sb[:, t, :], ps_t[:])

            # ---- C+D interleaved: scores per j-half, then that half's ---
            # ---- w accumulation (fills the PE gap while the next xT  ----
            # ---- chunks stream in)                                   ----
            sc_sb = wp.tile([HPC, S], BF16, name="sc_sb", tag="udw")
            eT_sb = wp.tile([P, JT, HPC], BF16, name="eT_sb")
            z2_sb = wp.tile([HPC, 2], FP32, name="z2_sb")
            w_sb = wp.tile([HPC, D], BF16, name="w_sb", tag="udw")
            ps_w = [psW.tile([HPC, JC], FP32, name=f"ps_w{oc}", tag=f"psW{oc}")
                    for oc in range(NJC)]

            for half in range(2):
                for jc in (2 * half, 2 * half + 1):
                    ps_s = psA.tile([HPC, JC], FP32, name="ps_s", tag="psA")
                    for t in range(DT):
                        nc.tensor.matmul(
                            ps_s[:],
                            lhsT=uT_sb[:, t, :],
                            rhs=xtc_sb[jc][:, t, :],
                            start=(t == 0),
                            stop=(t == DT - 1),
                        )
                    nc.vector.tensor_copy(
                        sc_sb[:, jc * JC:(jc + 1) * JC], ps_s[:])
                for lt in range(JT // 2):
                    t = half * (JT // 2) + lt
                    ps_e = psB.tile([P, HPC], BF16, name="ps_e", tag="psB")
                    nc.tensor.transpose(
                        ps_e[:], sc_sb[:, t * P:(t + 1) * P], ident_sb[:]
                    )
                    # exp folded into the PSUM evacuation (scalar ACT)
                    nc.scalar.activation(
                        eT_sb[:, t, :], ps_e[:],
                        mybir.ActivationFunctionType.Exp, scale=float(ISCALE),
                    )
                # softmax denominator for this half via exp+accum over the
                # raw scores; feeds only rz, which is consumed at the ctx
                # evacuation, so this sits OFF the w critical chain.
                nc.scalar.activation(
                    sc_sb[:, half * HJ:(half + 1) * HJ],
                    sc_sb[:, half * HJ:(half + 1) * HJ],
                    mybir.ActivationFunctionType.Exp, scale=float(ISCALE),
                    accum_out=z2_sb[:, half:half + 1],
                )
                # w accumulation: half 0 j-outer (overlaps the stream);
                # half 1 is emitted below, oc-outer, so chunks complete
                # sequentially and evac/wT/ctx pipeline behind them.
                if half == 0:
                    for JJ in range(8):
                        for oc in range(NJC):
                            nc.tensor.matmul(
                                ps_w[oc][:],
                                lhsT=eT_sb[:, JJ, :],
                                rhs=xnq_sb[JJ // QT][:, JJ % QT,
                                                     oc * JC:(oc + 1) * JC],
                                start=(JJ == 0),
                                stop=False,
                            )
            rz_sb = wp.tile([HPC, 1], FP32, name="rz_sb")
            z_sb = wp.tile([HPC, 1], FP32, name="z_sb")
            nc.vector.tensor_tensor(
                z_sb[:], z2_sb[:, 0:1], z2_sb[:, 1:2], mybir.AluOpType.add)
            nc.vector.reciprocal(rz_sb[:], z_sb[:])

            # ---- D+E: half-1 w chunks pipelined into wT and ctx ---------
            # w kept UNNORMALIZED (bf16 is scale-free); 1/z applied per-head
            # at the ctx evacuation, keeping exp/z/rz off the w chain.
            wT_sb = wp.tile([P, DT, HPC], BF16, name="wT_sb")
            cf_sb = wp.tile([HPC, HW], BF16, name="cf_sb")
            ctxT_sb = wp.tile([P, HPC, 1], BF16, name="ctxT_sb")
            ps_c = [psA.tile([HPC, 2 * HD], FP32, name=f"ps_c{pr}",
                             tag="psA") for pr in range(2)]
            for oc in range(NJC):
                for JJ in range(8, JT):
                    nc.tensor.matmul(
                        ps_w[oc][:],
                        lhsT=eT_sb[:, JJ, :],
                        rhs=xnq_sb[JJ // QT][:, JJ % QT,
                                             oc * JC:(oc + 1) * JC],
                        start=False,
                        stop=(JJ == JT - 1),
                    )
                if oc % 2 == 0:
                    nc.vector.tensor_copy(
                        w_sb[:, oc * JC:(oc + 1) * JC], ps_w[oc][:])
                else:
                    nc.scalar.activation(
                        w_sb[:, oc * JC:(oc + 1) * JC], ps_w[oc][:],
                        mybir.ActivationFunctionType.Copy,
                    )
                for lt in range(QT):
                    t = oc * QT + lt
                    ps_t = psB.tile([P, HPC], BF16, name="ps_wt", tag="psB")
                    nc.tensor.transpose(
                        ps_t[:], w_sb[:, t * P:(t + 1) * P], ident_sb[:]
                    )
                    nc.vector.tensor_copy(wT_sb[:, t, :], ps_t[:])
                for pr in range(2):
                    for lt in range(QT):
                        t = oc * QT + lt
                        nc.tensor.matmul(
                            ps_c[pr][:],
                            lhsT=wT_sb[:, t, :],
                            rhs=wv_sb[:, pr, t, :],
                            start=(t == 0),
                            stop=(t == DT - 1),
                        )
            for pr in range(2):
                if pr == 0:
                    nc.vector.tensor_scalar_mul(
                        cf_sb[:, pr * 2 * HD:(pr + 1) * 2 * HD], ps_c[pr][:],
                        rz_sb[:])
                else:
                    nc.scalar.activation(
                        cf_sb[:, pr * 2 * HD:(pr + 1) * 2 * HD], ps_c[pr][:],
                        mybir.ActivationFunctionType.Copy, scale=rz_sb[:],
                    )
                for hh in range(2):
                    h = pr * 2 + hh
                    ps_ct = psB.tile([P, HPC], BF16, name="ps_ct", tag="psB")
                    nc.tensor.transpose(
                        ps_ct[:], cf_sb[:, h * HD:(h + 1) * HD], ident_sb[:]
                    )
                    nc.vector.tensor_copy(ctxT_sb[:, h, :], ps_ct[:, h:h + 1])

            # ---- F: out per 512-chunk as each Wo chunk arrives ----------
            # bias added as a 5th accumulation step (e0 . bo rank-1 matmul)
            o_sb = wp.tile([1, D], FP32, name="o_sb")
            for oc in range(NJC):
                ps_o = psA.tile([1, JC], FP32, name="ps_o", tag="psA")
                for sub in range(HPC):
                    nc.tensor.matmul(
                        ps_o[:],
                        lhsT=ctxT_sb[:, sub, :],
                        rhs=wo_sb[:, oc, sub, :],
                        start=(sub == 0),
                        stop=False,
                    )
                nc.tensor.matmul(
                    ps_o[:],
                    lhsT=e0_sb[:],
                    rhs=bo_sb[:, oc * JC:(oc + 1) * JC],
                    start=False,
                    stop=True,
                )
                eng = nc.vector if oc % 2 == 0 else nc.scalar
                if oc % 2 == 0:
                    eng.tensor_copy(o_sb[:, oc * JC:(oc + 1) * JC], ps_o[:])
                else:
                    eng.activation(
                        o_sb[:, oc * JC:(oc + 1) * JC], ps_o[:],
                        mybir.ActivationFunctionType.Copy,
                    )
                nc.sync.dma_start(
                    out_sh[:, oc * JC:(oc + 1) * JC],
                    o_sb[:, oc * JC:(oc + 1) * JC],
                )

    nc.compile()
    return nc


_PROGRAM = None


def _get_program():
    global _PROGRAM
    if _PROGRAM is None:
        _PROGRAM = _build_program()
    return _PROGRAM


def _shard_inputs(x, Wq, Wk, Wv, Wo, bo):
    xb = x.astype(bfloat16)
    # w-path copy of x in fp8 E3M4 (cast from f32; |x| < 15.5 always for
    # N(0,1) data).  Errors here enter the output linearly (~1.2e-2 rel),
    # NOT through the softmax exponent, so fp8 is safe on this path only.
    x8 = x.astype(float8_e3m4)
    wqb = Wq.astype(bfloat16)
    wkb = Wk.astype(bfloat16)
    wvb = Wv.astype(bfloat16)
    wob = Wo.astype(bfloat16)
    bo4 = (bo / HPC).astype(bfloat16)
    identity = np.eye(HPC, dtype=bfloat16)

    in_maps = []
    for core in range(NC):
        b = core // HPC
        hg = core % HPC
        hs = slice(hg * HW, (hg + 1) * HW)
        xlastT_pre = np.ascontiguousarray(xb[b, -1, :].reshape(DT, P).T)
        wq_pre = np.ascontiguousarray(
            wqb[:, hs].reshape(DT, P, HW).transpose(1, 0, 2))
        wkT_pre = np.ascontiguousarray(
            wkb[:, hs].T.reshape(HPC, P, D).transpose(1, 0, 2))
        xT_pre = xb[b].T.reshape(DT, P, S).transpose(1, 0, 2)
        xn8_pre = x8[b].reshape(JT, P, D).transpose(1, 0, 2)
        wv_pre = wvb[:, hs].reshape(DT, P, HW).transpose(1, 0, 2)
        wo_pre = wob[hs, :].reshape(HPC, P, D).transpose(1, 0, 2)
        m = {
            "xlastT": xlastT_pre,
            "ident": identity,
            "wq": wq_pre,
            "wkT": wkT_pre,
            "bo_sh": bo4,
        }
        for c in range(NJC):
            m[f"xtc{c}"] = np.ascontiguousarray(
                xT_pre[:, :, c * JC:(c + 1) * JC])
        for i in range(4):
            m[f"xnq{i}"] = np.ascontiguousarray(
                xn8_pre[:, i * QT:(i + 1) * QT, :])
        # [P, DT, 256] head-pair column blocks for ctx streaming
        m["wvp0"] = np.ascontiguousarray(wv_pre[:, :, 0:2 * HD])
        m["wvp1"] = np.ascontiguousarray(wv_pre[:, :, 2 * HD:4 * HD])
        # [P, 2, HPC, JC] chunk-major pairs for per-chunk out streaming
        wo_cm = np.ascontiguousarray(
            wo_pre.reshape(P, HPC, NJC, JC).transpose(0, 2, 1, 3))
        m["wop0"] = np.ascontiguousarray(wo_cm[:, 0:2])
        m["wop1"] = np.ascontiguousarray(wo_cm[:, 2:4])
        in_maps.append(m)
    return in_maps


def kernel(x, Wq, Wk, Wv, Wo, bo, _trace=False, _trace_cores=None):
    x = np.asarray(x, dtype=np.float32)
    Wq = np.asarray(Wq, dtype=np.float32)
    Wk = np.asarray(Wk, dtype=np.float32)
    Wv = np.asarray(Wv, dtype=np.float32)
    Wo = np.asarray(Wo, dtype=np.float32)
    bo = np.asarray(bo, dtype=np.float32)

    nc = _get_program()
    in_maps = _shard_inputs(x, Wq, Wk, Wv, Wo, bo)
    res = run_bass_kernel_spmd(
        nc, in_maps, core_ids=list(range(NC)),
        trace=_trace, trace_cores=_trace_cores,
    )
    out = np.zeros((B, D), dtype=np.float32)
    for core in range(NC):
        out[core // HPC] += res.results[core]["out_sh"][0]
    if _trace:
        kernel._last_results = res
    return out
